# revision 22
# baseline (speedup 1.0000x reference)
"""Weighted Chamfer loss on Trainium2 (8 NeuronCores, batch-parallel).

Problem (per batch element b of 8):
    dist[i, j] = || set1[b, i] - set2[b, j] ||_2            (4096 x 4096, C=128)
    total = (sum_i w1[b,i] * min_j dist + sum_j w2[b,j] * min_i dist) / 2

Sharding: one batch element per NeuronCore (pure data parallel, no
collectives); the 8 per-core partial sums are added on the host.

Kernel strategy (vs the fp16 baseline):
  PE    : fp8e4 DoubleRow matmuls, K = 2 k-tiles x 66 rows = 128 channels
          + 4 spare rank-1 rows that bake -x2/2 (+ fp8 residual row) and
          -y2/2 (+ residual) into the same pass -> PSUM = -d^2/2 complete,
          at 0.5 cyc/row. One [128 x 4096] PSUM unit per x-block (all 8
          banks, 2-deep ping-pong), 8 matmuls of 512 cols each.
  ACT   : evacuates the unit with Identity(scale=-2) -> d2 fp16.
  DVE   : one fp16 tensor_reduce(min) over the unit -> row-min slot, and
          the column-min accumulator update (fp16 tensor_tensor min).
  Tail  : PE transposes of colacc + strided min reduce -> per-column mins;
          relu+sqrt; weighted sums; host adds the 8 per-core scalars.

The norms are computed from the *quantized* fp8 values (squares of fp8
are exact in fp16), so PSUM holds exactly -||x_q - y_q||^2/2 up to the
fp8 residual-row rounding (|err| <= ~0.25 on d2 values of ~170).
"""

import sys
from contextlib import ExitStack, nullcontext

import numpy as np

for _p in ("/opt/trn_rl_repo",):
    if _p not in sys.path:
        sys.path.insert(0, _p)

import concourse.bass as bass
import concourse.tile as tile
from concourse import bacc, masks, mybir
from concourse.bass_utils import run_bass_kernel_spmd

AF = mybir.ActivationFunctionType
ALU = mybir.AluOpType
DT = mybir.dt
PM = mybir.MatmulPerfMode

N_CORES = 8
N = 4096          # points per set per batch element
C = 128           # channels (contraction dim)
KP = C // 2 + 2   # 66: contraction rows per DoubleRow k-tile (64 ch + 2 bake)
NB = N // 128     # 32 row blocks of x
MMN = 512         # moving free dim per matmul (one fp32 PSUM bank)
NT = N // 128     # 32 transpose tiles

_CACHE = {}
LAST_RESULTS = None  # BassKernelResults of the most recent run (for profiling)


def _build_program(repeat=1, parts="pe,act,dve"):
    # tuning knob: "eNNNN" = ACT evac columns (rest go to DVE tensor_scalar)
    EA = N
    for p in parts.split(","):
        if p.startswith("e") and p[1:].isdigit():
            EA = int(p[1:])
    en_act = "act" in parts
    en_dve = "dve" in parts

    nc = bacc.Bacc(
        "TRN2", debug=False, target_bir_lowering=False, num_devices=N_CORES
    )
    xt_d = nc.dram_tensor("xt", [C, N], DT.float32, kind="ExternalInput").ap()
    yt_d = nc.dram_tensor("yt", [C, N], DT.float32, kind="ExternalInput").ap()
    # raw row squared mins + the [128, N] column accumulator; the final
    # partition-axis column min, relu, sqrt and weighting happen on the host
    out_d = nc.dram_tensor("out", [128, NB], DT.float32, kind="ExternalOutput").ap()
    oc_d = nc.dram_tensor("outc", [128, N], DT.float16, kind="ExternalOutput").ap()

    with tile.TileContext(nc) as tc, ExitStack() as ctx:
        persist = ctx.enter_context(tc.tile_pool(name="persist", bufs=1))
        prep = ctx.enter_context(tc.tile_pool(name="prep", bufs=1))
        rows = ctx.enter_context(tc.tile_pool(name="rows", bufs=1))
        d2p = ctx.enter_context(tc.tile_pool(name="d2", bufs=5))
        psum = ctx.enter_context(tc.tile_pool(name="psum", bufs=2, space="PSUM"))

        # ---------------- persistent tiles ----------------
        # DoubleRow operands [KP, 2, N] stored as [KP, 2*N]:
        #   x tile0 rows: ch 0..63, then -x2/2 (fp8), res_x (fp8)
        #   x tile1 rows: ch 64..127, then 1, 1
        #   y tile0 rows: ch 0..63, then 1, 1
        #   y tile1 rows: ch 64..127, then -y2/2, res_y
        xq8 = persist.tile([KP, 2 * N], DT.float8e4)
        yq8 = persist.tile([KP, 2 * N], DT.float8e4)

        ones = persist.tile([C, 1], DT.float16)
        nc.gpsimd.memset(ones[:], 1.0)

        colacc = persist.tile([128, N], DT.float16)
        nc.gpsimd.memset(colacc[:], 60000.0)

        rm = persist.tile([128, NB], DT.float32)

        # ---------------- prep: quantize + norms + assemble ----------------
        # spare rows default to 1.0; the norm rows overwrite their halves
        nc.vector.memset(xq8[C // 2 : KP, :], 1.0)
        nc.vector.memset(yq8[C // 2 : KP, :], 1.0)

        for src_d, q8big, half in ((xt_d, xq8, 0), (yt_d, yq8, 1)):
            stage = prep.tile([C, N], DT.float32, tag="stage")
            nc.sync.dma_start(stage[:], src_d[:])
            qfull = prep.tile([C, N], DT.float8e4, tag="qfull")
            nc.vector.tensor_copy(qfull[:], stage[:])
            h16 = prep.tile([C, N], DT.float16, tag="h16")
            nc.vector.tensor_copy(h16[:], qfull[:])
            sq = prep.tile([C, N], DT.float16, tag="sq")
            nc.scalar.activation(sq[:], h16[:], AF.Square)

            # -sum(sq)/2 over channels via ones-matmul into PSUM row 0
            mrow = rows.tile([1, N], DT.float32, tag="mrow")
            for hh in range(2):
                ps = psum.tile(
                    [128, N // 2], DT.float32, tag="unit", name=f"nps{half}{hh}"
                )
                for k in range(N // 2 // MMN):
                    c0 = k * MMN
                    nc.tensor.matmul(
                        ps[0:1, c0 : c0 + MMN],
                        ones[:],
                        sq[:, hh * (N // 2) + c0 : hh * (N // 2) + c0 + MMN],
                        start=True,
                        stop=True,
                    )
                nc.scalar.activation(
                    mrow[0:1, hh * (N // 2) : (hh + 1) * (N // 2)],
                    ps[0:1, :],
                    AF.Identity,
                    scale=-0.5,
                )

            m8 = rows.tile([1, N], DT.float8e4, tag="m8")
            nc.vector.tensor_copy(m8[:], mrow[:])
            mup = rows.tile([1, N], DT.float32, tag="mup")
            nc.vector.tensor_copy(mup[:], m8[:])
            resf = rows.tile([1, N], DT.float32, tag="resf")
            nc.vector.tensor_sub(resf[:], mrow[:], mup[:])
            r8 = rows.tile([1, N], DT.float8e4, tag="r8")
            nc.vector.tensor_copy(r8[:], resf[:])

            # assemble the [KP, 2, N] operand (SBUF->SBUF DMAs)
            nc.sync.dma_start(q8big[0 : C // 2, 0:N], qfull[0 : C // 2, :])
            nc.sync.dma_start(q8big[0 : C // 2, N : 2 * N], qfull[C // 2 : C, :])
            off = 0 if half == 0 else N
            nc.sync.dma_start(q8big[C // 2 : C // 2 + 1, off : off + N], m8[:])
            nc.sync.dma_start(q8big[C // 2 + 1 : KP, off : off + N], r8[:])

        xv = xq8[:].rearrange("p (two n) -> p two n", two=2)
        yv = yq8[:].rearrange("p (two n) -> p two n", two=2)

        if not en_dve:
            nc.gpsimd.memset(rm[:], 1.0)

        with tc.For_i(0, repeat, 1) if repeat > 1 else nullcontext():
            # ------- main loop: per x-block, two [128, 2048] PSUM units ------
            # evacuated into one [128, 4096] fp16 tile. All DVE ops use dense
            # unit-stride APs (strided operands lose the fp16 2x acceleration
            # on hardware). Folds run at 2x while tensor_reduce is 1x, so the
            # row min folds deep (to w=128) before the final small reduce.
            for b in range(NB):
                d2 = d2p.tile([128, N], DT.float16, tag="d2")
                for h in range(2):
                    hc = h * (N // 2)
                    ps = psum.tile([128, N // 2], DT.float32, tag="unit")
                    for k in range(N // 2 // MMN):
                        c0 = k * MMN
                        nc.tensor.matmul(
                            ps[:, c0 : c0 + MMN],
                            xv[:, :, b * 128 : (b + 1) * 128],
                            yv[:, :, hc + c0 : hc + c0 + MMN],
                            start=True,
                            stop=True,
                            perf_mode=PM.DoubleRow,
                        )
                    if en_act:
                        ea = max(0, min(EA - hc, N // 2))
                        if ea > 0:
                            nc.scalar.activation(
                                d2[:, hc : hc + ea],
                                ps[:, 0:ea],
                                AF.Identity,
                                scale=-2.0,
                            )
                        if ea < N // 2:
                            nc.vector.tensor_scalar_mul(
                                d2[:, hc + ea : hc + N // 2],
                                ps[:, ea : N // 2],
                                -2.0,
                            )
                if en_dve:
                    # col accumulator first (reads full d2), then the in-place
                    # row-min fold chain
                    nc.vector.tensor_tensor(
                        colacc[:], d2[:], colacc[:], ALU.min
                    )
                    w = N // 2
                    while w >= 512:
                        nc.vector.tensor_tensor(
                            d2[:, 0:w], d2[:, 0:w], d2[:, w : 2 * w], ALU.min
                        )
                        w //= 2
                    w *= 2
                    nc.vector.tensor_reduce(
                        rm[:, b : b + 1],
                        d2[:, 0:w].rearrange("p (t c) -> p t c", c=min(w, 512)),
                        axis=mybir.AxisListType.XY,
                        op=ALU.min,
                    )

            # ship raw row sq-mins + the whole column accumulator; the host
            # does the final partition-axis column min + relu+sqrt+weighting.
            # The 1MB colacc DMA overlaps the next iteration's pipeline fill.
            nc.sync.dma_start(out_d[:], rm[:])
            nc.sync.dma_start(oc_d[:], colacc[:])

    nc.compile()
    return nc


def _get_nc(repeat=1, parts="pe,act,dve"):
    key = ("nc", repeat, parts)
    if key not in _CACHE:
        _CACHE[key] = _build_program(repeat, parts)
    return _CACHE[key]


def _make_in_maps(set1, set2, w1, w2):
    in_maps = []
    for b in range(N_CORES):
        in_maps.append(
            {
                "xt": np.ascontiguousarray(set1[b].T, dtype=np.float32),
                "yt": np.ascontiguousarray(set2[b].T, dtype=np.float32),
            }
        )
    return in_maps


def kernel(set1, set2, w1, w2):
    global LAST_RESULTS
    set1 = np.asarray(set1, dtype=np.float32)
    set2 = np.asarray(set2, dtype=np.float32)
    w1 = np.asarray(w1, dtype=np.float32)
    w2 = np.asarray(w2, dtype=np.float32)

    nc = _get_nc()
    in_maps = _make_in_maps(set1, set2, w1, w2)
    res = run_bass_kernel_spmd(nc, in_maps, core_ids=list(range(N_CORES)))
    LAST_RESULTS = res

    # out[p, b] = row sq-min of row i = b*128+p; outc[p, j] = column
    # accumulator (min over partition axis p gives col j's sq-min).
    # relu+sqrt+weighted sum on host.
    total = 0.0
    for core, core_out in enumerate(res.results):
        rowmin = core_out["out"].astype(np.float64).T.reshape(-1)  # b-major
        colmin = core_out["outc"].astype(np.float64).min(axis=0)   # per col j
        dr = np.sqrt(np.maximum(rowmin, 0.0))
        dc = np.sqrt(np.maximum(colmin, 0.0))
        total += (w1[core].astype(np.float64) * dr).sum()
        total += (w2[core].astype(np.float64) * dc).sum()
    return np.float32(total / 2.0)
